# revision 1
# baseline (speedup 1.0000x reference)
"""Trainium2 Bass kernel for nn_Decoder (GNN message passing):
LSTM(1 step) -> GCNConv -> ReLU -> GCNConv -> Linear -> ReLU on a
100K-node / 1.6M-edge graph, SPMD across 8 NeuronCores.

Strategy (dst-node sharding):
- Core c owns nodes [c*12500, (c+1)*12500) and all edges into them.
- Per-node compute (LSTM, x@W transforms) runs feature-major [128, nodes]
  so all matmuls need zero transposes and biases are per-partition.
- The GCN propagate gathers transformed rows from a bf16 node-major table
  in DRAM (built via AllGather of the 8 shards) with gpsimd.dma_gather,
  then scatter-adds via PE matmul with an on-chip selection matrix
  (tensor_scalar: iota==dst_idx -> * norm), accumulated in PSUM per
  128-dst block.
"""

from contextlib import ExitStack

import numpy as np
import ml_dtypes

import concourse.bacc as bacc
import concourse.mybir as mybir
import concourse.tile as tile
from concourse.bass_utils import run_bass_kernel_spmd

P = 128
N = 100000
NCORES = 8
NPC = N // NCORES            # 12500 nodes per core
NBLK = (NPC + P - 1) // P    # 98 dst blocks per core (last has 84)
CH = 4                       # src chunks (int16 gather index limit)
QROWS = NPC // CH            # 3125: per-rank quarter contributed to a chunk
CHROWS = QROWS * NCORES      # 25000 rows per chunk table
GT = 48                      # tiles (of 128 edges) per dma_gather
LSTM_CHUNK = 500             # nodes per LSTM/matmul column chunk

bf16 = ml_dtypes.bfloat16
f32 = np.float32


# ---------------------------------------------------------------- host prep


def _prep_edges(edge_index):
    """Sort/pad each core's incident edges into a cross-core-uniform tile
    schedule. Returns per-core device arrays + the static schedule."""
    src = np.asarray(edge_index[0], dtype=np.int64)
    dst = np.asarray(edge_index[1], dtype=np.int64)
    loops = np.arange(N, dtype=np.int64)
    src = np.concatenate([src, loops])
    dst = np.concatenate([dst, loops])

    deg = np.bincount(dst, minlength=N).astype(np.float64)
    dinv = 1.0 / np.sqrt(deg)
    norm = (dinv[src] * dinv[dst]).astype(np.float32)

    core_of = dst // NPC
    per_core = []
    counts = np.zeros((NCORES, CH, NBLK), np.int64)
    for c in range(NCORES):
        m = core_of == c
        s = src[m]
        d = dst[m] - c * NPC
        w = norm[m]
        ch = (s % NPC) // QROWS
        o = np.lexsort((d, ch))
        s, d, w, ch = s[o], d[o], w[o], ch[o]
        b = d // P
        counts[c] = np.bincount(ch * NBLK + b, minlength=CH * NBLK).reshape(
            CH, NBLK
        )
        per_core.append((s, d, w, ch, b))

    # tiles per (chunk, block) run: padded to the max across cores
    T_run = (counts.max(axis=0) + P - 1) // P          # [CH, NBLK]
    flat = T_run.reshape(-1)
    base = np.zeros(CH * NBLK + 1, np.int64)
    np.cumsum(flat, out=base[1:])                      # tile offset per run
    TT = int(base[-1])
    NIDX = TT * P
    ctb = [int(base[ch * NBLK]) for ch in range(CH)] + [TT]  # chunk tile base

    arrs = []
    for c in range(NCORES):
        s, d, w, ch, b = per_core[c]
        gid = ch * NBLK + b
        cnt = counts[c].reshape(-1)
        gstart = np.concatenate([[0], np.cumsum(cnt)[:-1]])
        within = np.arange(len(s)) - gstart[gid]
        pos = base[gid] * P + within

        idxs = np.zeros(NIDX, np.int16)                 # pad -> row 0 (valid)
        # chunk q table = concat over ranks of each rank's q-th quarter
        idxs[pos] = ((s // NPC) * QROWS + (s % QROWS)).astype(np.int16)
        dstv = np.full(NIDX, -1.0, np.float32)          # pad -> no dst match
        dstv[pos] = (d - b * P).astype(np.float32)
        nrmv = np.zeros(NIDX, np.float32)
        nrmv[pos] = w

        idx16 = np.tile(np.ascontiguousarray(idxs.reshape(-1, 16).T), (8, 1))
        dstt = np.ascontiguousarray(dstv.reshape(TT, P).T)
        nrmt = np.ascontiguousarray(nrmv.reshape(TT, P).T)
        arrs.append((idx16, dstt, nrmt))

    # gather pieces: per chunk, consecutive groups of <= GT tiles
    pieces = []
    for chn in range(CH):
        t0, t1 = ctb[chn], ctb[chn + 1]
        pieces.append([(t, min(GT, t1 - t)) for t in range(t0, t1, GT)])

    sched = dict(T_run=T_run, base=base, TT=TT, NIDX=NIDX, ctb=ctb, pieces=pieces)
    return arrs, sched


# ---------------------------------------------------------------- device


def _build_nc(sched):
    T_run, base, TT, NIDX, ctb, pieces = (
        sched["T_run"],
        sched["base"],
        sched["TT"],
        sched["NIDX"],
        sched["ctb"],
        sched["pieces"],
    )
    dt = mybir.dt
    alu = mybir.AluOpType
    act = mybir.ActivationFunctionType

    nc = bacc.Bacc("TRN2", target_bir_lowering=False, debug=False, num_devices=NCORES)

    # ---- I/O
    zT_d = nc.dram_tensor("zT", [P, NPC], dt.bfloat16, kind="ExternalInput")
    idx_d = nc.dram_tensor("idx16", [P, NIDX // 16], dt.int16, kind="ExternalInput")
    dst_d = nc.dram_tensor("dstv", [P, TT], dt.float32, kind="ExternalInput")
    nrm_d = nc.dram_tensor("nrmv", [P, TT], dt.float32, kind="ExternalInput")
    iota_d = nc.dram_tensor("iota", [P, P], dt.bfloat16, kind="ExternalInput")
    wih_d = {
        g: nc.dram_tensor(f"wih_{g}", [P, P], dt.bfloat16, kind="ExternalInput")
        for g in "igo"
    }
    bg_d = {
        g: nc.dram_tensor(f"bg_{g}", [P, 1], dt.float32, kind="ExternalInput")
        for g in "igo"
    }
    w1_d = nc.dram_tensor("w1", [P, P], dt.bfloat16, kind="ExternalInput")
    w2_d = nc.dram_tensor("w2", [P, P], dt.bfloat16, kind="ExternalInput")
    w3t_d = nc.dram_tensor("w3t", [P, P], dt.bfloat16, kind="ExternalInput")
    b1_d = nc.dram_tensor("b1", [P, 1], dt.float32, kind="ExternalInput")
    b2_d = nc.dram_tensor("b2", [P, 1], dt.float32, kind="ExternalInput")
    b3_d = nc.dram_tensor("b3", [P, 1], dt.float32, kind="ExternalInput")
    out_d = nc.dram_tensor("outT", [P, NPC], dt.float32, kind="ExternalOutput")

    bounce = [nc.dram_tensor(f"bounce{l}", [NPC, P], dt.bfloat16) for l in range(2)]
    table = [
        [nc.dram_tensor(f"table{l}_{q}", [CHROWS, P], dt.bfloat16) for q in range(CH)]
        for l in range(2)
    ]

    with tile.TileContext(nc) as tc, ExitStack() as ctx:
        konst = ctx.enter_context(tc.tile_pool(name="konst", bufs=1))
        big = ctx.enter_context(tc.tile_pool(name="big", bufs=1))

        def load_const(handle, shape, dtype):
            t = konst.tile(shape, dtype, tag=handle.name)
            nc.sync.dma_start(t[:], handle[:])
            return t

        iota_t = load_const(iota_d, [P, P], dt.bfloat16)
        wih_t = {g: load_const(wih_d[g], [P, P], dt.bfloat16) for g in "igo"}
        bg_t = {g: load_const(bg_d[g], [P, 1], dt.float32) for g in "igo"}
        w1_t = load_const(w1_d, [P, P], dt.bfloat16)
        w2_t = load_const(w2_d, [P, P], dt.bfloat16)
        w3t_t = load_const(w3t_d, [P, P], dt.bfloat16)
        b1_t = load_const(b1_d, [P, 1], dt.float32)
        b2_t = load_const(b2_d, [P, 1], dt.float32)
        b3_t = load_const(b3_d, [P, 1], dt.float32)
        idx_t = load_const(idx_d, [P, NIDX // 16], dt.int16)
        dst_t = load_const(dst_d, [P, TT], dt.float32)
        nrm_t = load_const(nrm_d, [P, TT], dt.float32)

        xT_t = big.tile([P, NPC], dt.bfloat16, tag="xT")  # x1T then x2T

        # ---------------- phase 1: LSTM -> hT (feature-major, bf16)
        with tc.tile_pool(name="h_pool", bufs=1) as hpool:
            hT_t = hpool.tile([P, NPC], dt.bfloat16, tag="hT")
            with (
                tc.tile_pool(name="lstm_sb", bufs=1) as lsb,
                tc.tile_pool(name="lstm_ps", bufs=6, space="PSUM") as lps,
                tc.tile_pool(name="lstm_tr", bufs=8) as ltr,
            ):
                zT_t = lsb.tile([P, NPC], dt.bfloat16, tag="zT")
                nc.sync.dma_start(zT_t[:], zT_d[:])

                nchunk = (NPC + LSTM_CHUNK - 1) // LSTM_CHUNK
                for k in range(nchunk):
                    c0 = k * LSTM_CHUNK
                    c1 = min(NPC, c0 + LSTM_CHUNK)
                    w = c1 - c0
                    gate = {}
                    for g in "igo":
                        ps = lps.tile([P, LSTM_CHUNK], dt.float32, tag="ps")
                        nc.tensor.matmul(
                            ps[:, :w], wih_t[g][:], zT_t[:, c0:c1], start=True, stop=True
                        )
                        fn = act.Tanh if g == "g" else act.Sigmoid
                        sg = ltr.tile([P, LSTM_CHUNK], dt.bfloat16, tag="sg" + g)
                        nc.scalar.activation(sg[:, :w], ps[:, :w], fn, bias=bg_t[g][:])
                        gate[g] = sg
                    ct = ltr.tile([P, LSTM_CHUNK], dt.bfloat16, tag="ct")
                    nc.vector.tensor_tensor(
                        ct[:, :w], gate["i"][:, :w], gate["g"][:, :w], op=alu.mult
                    )
                    th = ltr.tile([P, LSTM_CHUNK], dt.bfloat16, tag="th")
                    nc.scalar.activation(th[:, :w], ct[:, :w], act.Tanh)
                    nc.vector.tensor_tensor(
                        hT_t[:, c0:c1], gate["o"][:, :w], th[:, :w], op=alu.mult
                    )

            # ---------------- phase 2: m1 = (h @ W1) node-major -> bounce0
            _mm_to_bounce(nc, tc, hT_t, w1_t, bounce[0])

        _allgather(nc, bounce[0], table[0])

        with (
            tc.tile_pool(name="stag", bufs=6) as stag,
            tc.tile_pool(name="spool", bufs=12) as spool,
        ):
            # ------------- phase 3: edge layer 1 -> x1T = relu(agg + b1)
            def post1(b, nb, pa):
                nc.scalar.activation(
                    xT_t[:, b * P : b * P + nb], pa[:, :nb], act.Relu, bias=b1_t[:]
                )

            _edge_phase(nc, tc, table[0], sched, idx_t, dst_t, nrm_t, iota_t, stag, spool, post1)

            # ------------- phase 4: m2 = (x1 @ W2) node-major -> bounce1
            _mm_to_bounce(nc, tc, xT_t, w2_t, bounce[1])
            _allgather(nc, bounce[1], table[1])

            # ------------- phase 5: edge layer 2 -> x2T = agg + b2 (no relu)
            def post2(b, nb, pa):
                nc.vector.tensor_scalar(
                    xT_t[:, b * P : b * P + nb], pa[:, :nb], b2_t[:], None, op0=alu.add
                )

            _edge_phase(nc, tc, table[1], sched, idx_t, dst_t, nrm_t, iota_t, stag, spool, post2)

        # ---------------- phase 6: outT = relu(W3T.T @ x2T + b3)
        with (
            tc.tile_pool(name="out_ps", bufs=3, space="PSUM") as ops,
            tc.tile_pool(name="out_sb", bufs=3) as osb,
        ):
            nchunk = (NPC + LSTM_CHUNK - 1) // LSTM_CHUNK
            for k in range(nchunk):
                c0 = k * LSTM_CHUNK
                c1 = min(NPC, c0 + LSTM_CHUNK)
                w = c1 - c0
                ps = ops.tile([P, LSTM_CHUNK], dt.float32, tag="ps")
                nc.tensor.matmul(
                    ps[:, :w], w3t_t[:], xT_t[:, c0:c1], start=True, stop=True
                )
                ot = osb.tile([P, LSTM_CHUNK], dt.float32, tag="ot")
                nc.scalar.activation(ot[:, :w], ps[:, :w], act.Relu, bias=b3_t[:])
                nc.sync.dma_start(out_d[:, c0:c1], ot[:, :w])

    nc.compile()
    return nc


def _mm_to_bounce(nc, tc, featT, w_t, bounce_d):
    """Per 128-node block: matmul(lhsT=featT block, rhs=W) -> node-major
    [node, feat] psum -> bf16 stage -> one strided DMA into bounce DRAM."""
    dt = mybir.dt
    act = mybir.ActivationFunctionType
    with (
        tc.tile_pool(name="m_ps", bufs=2, space="PSUM") as mps,
        tc.tile_pool(name="m_sb", bufs=1) as msb,
    ):
        stage = msb.tile([P, NBLK * P], dt.bfloat16, tag="mstage")
        for b in range(NBLK):
            nb = min(P, NPC - b * P)
            pm = mps.tile([P, P], dt.float32, tag="pm")
            nc.tensor.matmul(
                pm[:nb, :], featT[:, b * P : b * P + nb], w_t[:], start=True, stop=True
            )
            nc.scalar.activation(
                stage[:nb, b * P : (b + 1) * P], pm[:nb, :], act.Copy
            )
        full = (NPC // P) * P  # 12416
        nc.sync.dma_start(
            bounce_d[:full, :].rearrange("(b p) f -> p b f", p=P),
            stage[:, : NPC // P * P].rearrange("p (b f) -> p b f", f=P),
        )
        rem = NPC - full
        if rem:
            nc.sync.dma_start(bounce_d[full:, :], stage[:rem, full:])


def _allgather(nc, bounce_d, tables_d):
    # one sub-AllGather per quarter: output q IS chunk table q (offset-free),
    # and chunk-q edge gathers can start as soon as AG#q lands.
    for q in range(CH):
        nc.gpsimd.collective_compute(
            "AllGather",
            mybir.AluOpType.bypass,
            replica_groups=[list(range(NCORES))],
            ins=[bounce_d[q * QROWS : (q + 1) * QROWS, :]],
            outs=[tables_d[q][:]],
        )


def _edge_phase(nc, tc, table_d, sched, idx_t, dst_t, nrm_t, iota_t, stag, spool, post):
    dt = mybir.dt
    alu = mybir.AluOpType
    T_run, base, ctb, pieces = (
        sched["T_run"],
        sched["base"],
        sched["ctb"],
        sched["pieces"],
    )
    piece_tiles = {}
    with tc.tile_pool(name="agg_ps", bufs=6, space="PSUM") as aps:
        for b in range(NBLK):
            nb = min(P, NPC - b * P)
            pa = aps.tile([P, P], dt.float32, tag="pa")
            ntile_b = int(T_run[:, b].sum())
            done = 0
            for chn in range(CH):
                for t in range(int(T_run[chn][b])):
                    gt = int(base[chn * NBLK + b]) + t
                    rel = gt - ctb[chn]
                    pi, slot = divmod(rel, GT)
                    key = (chn, pi)
                    if key not in piece_tiles:
                        pt0, pnt = pieces[chn][pi]
                        stg = stag.tile([P, GT, P], dt.bfloat16, tag="stag")
                        nc.gpsimd.dma_gather(
                            stg[:, :pnt, :],
                            table_d[chn][:],
                            idx_t[:, pt0 * 8 : (pt0 + pnt) * 8],
                            pnt * P,
                            pnt * P,
                            P,
                            single_packet=False,
                        )
                        piece_tiles[key] = stg
                    stg = piece_tiles[key]
                    st = spool.tile([P, P], dt.bfloat16, tag="st")
                    nc.vector.tensor_scalar(
                        st[:],
                        iota_t[:],
                        dst_t[:, gt : gt + 1],
                        nrm_t[:, gt : gt + 1],
                        op0=alu.is_equal,
                        op1=alu.mult,
                    )
                    nc.tensor.matmul(
                        pa[:],
                        stg[:, slot, :],
                        st[:],
                        start=(done == 0),
                        stop=(done == ntile_b - 1),
                    )
                    done += 1
            post(b, nb, pa)


# ---------------------------------------------------------------- entry


def build(z, edge_index, W_ih, W_hh, b_ih, b_hh, W1, b1, W2, b2, W3, b3):
    """Host prep + trace + compile. Returns (nc, in_maps)."""
    z = np.asarray(z, dtype=np.float32)
    W_ih = np.asarray(W_ih, dtype=np.float32)
    b = np.asarray(b_ih, dtype=np.float32) + np.asarray(b_hh, dtype=np.float32)

    arrs, sched = _prep_edges(edge_index)
    nc = _build_nc(sched)

    gi = {"i": 0, "g": 2, "o": 3}  # torch gate order i,f,g,o (f unused: c0=0)
    common = {
        "iota": np.ascontiguousarray(
            np.tile(np.arange(P, dtype=np.float32), (P, 1))
        ).astype(bf16),
        "w1": np.asarray(W1, np.float32).astype(bf16),
        "w2": np.asarray(W2, np.float32).astype(bf16),
        "w3t": np.ascontiguousarray(np.asarray(W3, np.float32).T).astype(bf16),
        "b1": np.asarray(b1, np.float32).reshape(P, 1).copy(),
        "b2": np.asarray(b2, np.float32).reshape(P, 1).copy(),
        "b3": np.asarray(b3, np.float32).reshape(P, 1).copy(),
    }
    for g, k in gi.items():
        common[f"wih_{g}"] = np.ascontiguousarray(
            W_ih[k * P : (k + 1) * P, :].T
        ).astype(bf16)
        common[f"bg_{g}"] = b[k * P : (k + 1) * P].reshape(P, 1).copy()

    in_maps = []
    for c in range(NCORES):
        idx16, dstt, nrmt = arrs[c]
        m = dict(common)
        m["zT"] = np.ascontiguousarray(z[c * NPC : (c + 1) * NPC].T).astype(bf16)
        m["idx16"] = idx16
        m["dstv"] = dstt
        m["nrmv"] = nrmt
        in_maps.append(m)
    return nc, in_maps


def assemble(results):
    out = np.empty((N, P), np.float32)
    for c in range(NCORES):
        out[c * NPC : (c + 1) * NPC] = results[c]["outT"].T
    return out


def kernel(z, edge_index, W_ih, W_hh, b_ih, b_hh, W1, b1, W2, b2, W3, b3):
    nc, in_maps = build(z, edge_index, W_ih, W_hh, b_ih, b_hh, W1, b1, W2, b2, W3, b3)
    res = run_bass_kernel_spmd(nc, in_maps, core_ids=list(range(NCORES)))
    return assemble(res.results)



# revision 11
# speedup vs baseline: 2.5476x; 2.5476x over previous
"""Trainium2 Bass kernel for nn_Decoder (GNN message passing):
LSTM(1 step) -> GCNConv -> ReLU -> GCNConv -> Linear -> ReLU on a
100K-node / 1.6M-edge graph, SPMD across 8 NeuronCores.

Strategy (dst-node sharding, v2):
- Core c owns nodes [c*12500, (c+1)*12500) and all edges into them.
- Per-node compute (LSTM, x@W transforms) runs feature-major [128, nodes].
- The transformed node table is replicated across cores via per-quarter
  AllGathers into 4 DRAM chunk tables (<=32768 rows each: int16 gather
  index limit); quarters are 128-block-aligned so the AG input is a
  contiguous slice of the node-major bounce buffer.
- GCN propagate: per dst 128-block, gather source rows (bf16) from the
  chunk tables with gpsimd.dma_gather (in-order per-chunk piece streams,
  prefetched), scatter-add via PE matmul against 128x128 selection
  matrices, accumulated in PSUM.
- Selection matrices are built in bulk, one gather piece at a time, with
  two broadcast tensor_tensor ops: st = (iota == dstv) * nrm.
- Per-block interleaving: x@W2 runs inside edge layer 1 (quarters AG as
  soon as ready); the final Linear runs inside edge layer 2 with grouped
  output DMAs.
"""

from contextlib import ExitStack

import numpy as np
import ml_dtypes

import concourse.bacc as bacc
import concourse.mybir as mybir
import concourse.tile as tile
from concourse.bass_utils import run_bass_kernel_spmd

P = 128
N = 100000
NCORES = 8
CH = 4              # src quarters (chunk tables)
GT = 18             # tiles (of 128 edges) per gather piece
LSTM_CHUNK = 500    # nodes per LSTM column chunk
GBO = 14            # dst blocks per output DMA group

bf16 = ml_dtypes.bfloat16
f32 = np.float32


def _cfg(n=N, ncores=NCORES):
    npc = n // ncores
    nblk = (npc + P - 1) // P
    qb = (nblk + CH - 1) // CH
    qb0 = [min(nblk, q * qb) for q in range(CH + 1)]   # quarter block starts
    q0 = [min(npc, b * P) for b in qb0]                # quarter node starts
    qs = [q0[q + 1] - q0[q] for q in range(CH)]        # quarter sizes (nodes)
    assert all(0 < ncores * s <= 32768 for s in qs)
    return dict(n=n, ncores=ncores, npc=npc, nblk=nblk, qb0=qb0, q0=q0, qs=qs)


# ---------------------------------------------------------------- host prep


def _prep_edges(edge_index, cfg):
    """Sort/pad each core's incident edges into a cross-core-uniform tile
    schedule. Returns per-core device arrays + the static schedule."""
    n, ncores, npc, nblk = cfg["n"], cfg["ncores"], cfg["npc"], cfg["nblk"]
    q0, qs = cfg["q0"], cfg["qs"]

    src = np.asarray(edge_index[0], dtype=np.int64)
    dst = np.asarray(edge_index[1], dtype=np.int64)
    loops = np.arange(n, dtype=np.int64)
    src = np.concatenate([src, loops])
    dst = np.concatenate([dst, loops])

    deg = np.bincount(dst, minlength=n).astype(np.float64)
    dinv = 1.0 / np.sqrt(deg)
    norm = (dinv[src] * dinv[dst]).astype(np.float32)

    # chunk (src quarter) + row in that chunk's table
    s_rank = src // npc
    s_local = src % npc
    s_ch = np.searchsorted(np.asarray(q0[1 : CH + 1]), s_local, side="right")
    qs_a = np.asarray(qs)
    q0_a = np.asarray(q0[:CH])
    s_row = s_rank * qs_a[s_ch] + (s_local - q0_a[s_ch])

    core_of = dst // npc
    per_core = []
    counts = np.zeros((ncores, CH, nblk), np.int64)
    for c in range(ncores):
        m = core_of == c
        row, d, w, ch = s_row[m], dst[m] - c * npc, norm[m], s_ch[m]
        o = np.lexsort((d, ch))
        row, d, w, ch = row[o], d[o], w[o], ch[o]
        b = d // P
        counts[c] = np.bincount(ch * nblk + b, minlength=CH * nblk).reshape(
            CH, nblk
        )
        per_core.append((row, d, w, ch, b))

    assert counts.sum(axis=(0, 1)).min() > 0, "empty dst block"

    T_run = (counts.max(axis=0) + P - 1) // P          # [CH, nblk]
    flat = T_run.reshape(-1)
    base = np.zeros(CH * nblk + 1, np.int64)
    np.cumsum(flat, out=base[1:])                      # tile offset per run
    TT = int(base[-1])
    NIDX = TT * P
    ctb = [int(base[ch * nblk]) for ch in range(CH)] + [TT]  # chunk tile base

    arrs = []
    for c in range(ncores):
        row, d, w, ch, b = per_core[c]
        gid = ch * nblk + b
        cnt = counts[c].reshape(-1)
        gstart = np.concatenate([[0], np.cumsum(cnt)[:-1]])
        within = np.arange(len(row)) - gstart[gid]
        pos = base[gid] * P + within

        idxs = np.zeros(NIDX, np.int16)                 # pad -> row 0 (valid)
        idxs[pos] = row.astype(np.int16)
        dstv = np.full(NIDX, -1.0, np.float32)          # pad -> no dst match
        dstv[pos] = (d - b * P).astype(np.float32)
        nrmv = np.zeros(NIDX, np.float32)
        nrmv[pos] = w

        idx16 = np.tile(np.ascontiguousarray(idxs.reshape(-1, 16).T), (8, 1))
        dstt = np.ascontiguousarray(dstv.reshape(TT, P).T).astype(bf16)
        nrmt = np.ascontiguousarray(nrmv.reshape(TT, P).T).astype(bf16)
        arrs.append((idx16, dstt, nrmt))

    # gather pieces: per chunk, consecutive groups of <= GT tiles
    pieces = []
    for chn in range(CH):
        t0, t1 = ctb[chn], ctb[chn + 1]
        pieces.append([(t, min(GT, t1 - t)) for t in range(t0, t1, GT)])

    sched = dict(T_run=T_run, base=base, TT=TT, NIDX=NIDX, ctb=ctb, pieces=pieces)
    return arrs, sched


# ---------------------------------------------------------------- device


def _build_nc(cfg, sched):
    ncores, npc, nblk = cfg["ncores"], cfg["npc"], cfg["nblk"]
    qb0, q0, qs = cfg["qb0"], cfg["q0"], cfg["qs"]
    T_run, base, TT, NIDX, ctb, pieces = (
        sched["T_run"],
        sched["base"],
        sched["TT"],
        sched["NIDX"],
        sched["ctb"],
        sched["pieces"],
    )
    dt = mybir.dt
    alu = mybir.AluOpType
    act = mybir.ActivationFunctionType

    nc = bacc.Bacc("TRN2", target_bir_lowering=False, debug=False, num_devices=ncores)

    # ---- I/O
    zT_d = nc.dram_tensor("zT", [P, npc], dt.bfloat16, kind="ExternalInput")
    idx_d = nc.dram_tensor("idx16", [P, NIDX // 16], dt.int16, kind="ExternalInput")
    dst_d = nc.dram_tensor("dstv", [P, TT], dt.bfloat16, kind="ExternalInput")
    nrm_d = nc.dram_tensor("nrmv", [P, TT], dt.bfloat16, kind="ExternalInput")
    iota_d = nc.dram_tensor("iota", [P, P], dt.bfloat16, kind="ExternalInput")
    wih_d = {
        g: nc.dram_tensor(f"wih_{g}", [P, P], dt.bfloat16, kind="ExternalInput")
        for g in "igo"
    }
    bg_d = {
        g: nc.dram_tensor(f"bg_{g}", [P, 1], dt.float32, kind="ExternalInput")
        for g in "igo"
    }
    w1_d = nc.dram_tensor("w1", [P, P], dt.bfloat16, kind="ExternalInput")
    w2_d = nc.dram_tensor("w2", [P, P], dt.bfloat16, kind="ExternalInput")
    w3t_d = nc.dram_tensor("w3t", [P, P], dt.bfloat16, kind="ExternalInput")
    b1_d = nc.dram_tensor("b1", [P, 1], dt.float32, kind="ExternalInput")
    b2_d = nc.dram_tensor("b2", [P, 1], dt.float32, kind="ExternalInput")
    b3_d = nc.dram_tensor("b3", [P, 1], dt.float32, kind="ExternalInput")
    out_d = nc.dram_tensor("outT", [P, npc], dt.float32, kind="ExternalOutput")

    bounce = [nc.dram_tensor(f"bounce{l}", [npc, P], dt.bfloat16) for l in range(2)]
    table = [
        [
            nc.dram_tensor(f"table{l}_{q}", [ncores * qs[q], P], dt.bfloat16)
            for q in range(CH)
        ]
        for l in range(2)
    ]

    with tile.TileContext(nc) as tc, ExitStack() as ctx:
        konst = ctx.enter_context(tc.tile_pool(name="konst", bufs=1))
        xpool = ctx.enter_context(tc.tile_pool(name="xpool", bufs=1))
        spool = ctx.enter_context(tc.tile_pool(name="spool", bufs=1))

        def load_const(handle, shape, dtype):
            t = konst.tile(shape, dtype, tag=handle.name)
            nc.sync.dma_start(t[:], handle[:])
            return t

        iota_t = load_const(iota_d, [P, P], dt.bfloat16)
        wih_t = {g: load_const(wih_d[g], [P, P], dt.bfloat16) for g in "igo"}
        bg_t = {g: load_const(bg_d[g], [P, 1], dt.float32) for g in "igo"}
        w1_t = load_const(w1_d, [P, P], dt.bfloat16)
        w2_t = load_const(w2_d, [P, P], dt.bfloat16)
        w3t_t = load_const(w3t_d, [P, P], dt.bfloat16)
        b1_t = load_const(b1_d, [P, 1], dt.float32)
        b2_t = load_const(b2_d, [P, 1], dt.float32)
        b3_t = load_const(b3_d, [P, 1], dt.float32)
        idx_t = load_const(idx_d, [P, NIDX // 16], dt.int16)
        dst_t = load_const(dst_d, [P, TT], dt.bfloat16)
        nrm_t = load_const(nrm_d, [P, TT], dt.bfloat16)

        xT_t = xpool.tile([P, npc], dt.bfloat16, tag="xT")   # x1T then x2T
        stage = spool.tile([P, nblk * P], dt.bfloat16, tag="stage")

        # per-chunk gather-piece pools (double-buffered, in-order streams);
        # created after the LSTM scratch pools close, to fit SBUF
        stgp = [None] * CH
        selp = [None] * CH
        issued = [[-1] * CH for _ in range(2)]      # last piece issued, per layer
        ptiles = [[{} for _ in range(CH)] for _ in range(2)]

        def issue_piece(l, chn):
            k = issued[l][chn] + 1
            if k >= len(pieces[chn]):
                return
            pt0, pnt = pieces[chn][k]
            stg = stgp[chn].tile([P, GT, P], dt.bfloat16, tag=f"stg{chn}")
            nc.gpsimd.dma_gather(
                stg[:, :pnt, :],
                table[l][chn][:],
                idx_t[:, pt0 * 8 : (pt0 + pnt) * 8],
                pnt * P,
                pnt * P,
                P,
                single_packet=False,
            )
            st = selp[chn].tile([P, GT * P], dt.bfloat16, tag=f"sel{chn}")
            stv = st[:, : pnt * P].rearrange("p (t c) -> p t c", c=P)
            iotaB = iota_t[:].unsqueeze(1).broadcast_to([P, pnt, P])
            dstvB = (
                dst_t[:, pt0 : pt0 + pnt].unsqueeze(2).broadcast_to([P, pnt, P])
            )
            nrmB = (
                nrm_t[:, pt0 : pt0 + pnt].unsqueeze(2).broadcast_to([P, pnt, P])
            )
            nc.vector.tensor_tensor(stv, iotaB, dstvB, op=alu.is_equal)
            nc.vector.tensor_tensor(stv, stv, nrmB, op=alu.mult)
            ptiles[l][chn][k] = (stg, st)
            issued[l][chn] = k

        def quarter_flush(l, q):
            """DMA stage quarter q -> bounce[l], then AllGather the quarter."""
            r0, r1 = q0[q], q0[q + 1]
            full = ((r1 - r0) // P) * P
            if full:
                nc.sync.dma_start(
                    bounce[l][r0 : r0 + full, :].rearrange("(b p) f -> p b f", p=P),
                    stage[:, r0 : r0 + full].rearrange("p (b f) -> p b f", f=P),
                )
            rem = (r1 - r0) - full
            if rem:
                nc.sync.dma_start(
                    bounce[l][r0 + full : r1, :],
                    stage[:rem, r0 + full : r0 + full + P],
                )
            nc.gpsimd.collective_compute(
                "AllGather",
                mybir.AluOpType.bypass,
                replica_groups=[list(range(ncores))],
                ins=[bounce[l][r0:r1, :]],
                outs=[table[l][q][:]],
            )

        # ---------------- phase 1: LSTM -> hT (feature-major, bf16)
        hpool = ctx.enter_context(tc.tile_pool(name="h_pool", bufs=1))
        hT_t = hpool.tile([P, npc], dt.bfloat16, tag="hT")
        with (
            tc.tile_pool(name="lstm_sb", bufs=1) as lsb,
            tc.tile_pool(name="lstm_ps", bufs=6, space="PSUM") as lps,
            tc.tile_pool(name="lstm_tr", bufs=8) as ltr,
        ):
            zT_t = lsb.tile([P, npc], dt.bfloat16, tag="zT")
            nc.sync.dma_start(zT_t[:], zT_d[:])

            nchunk = (npc + LSTM_CHUNK - 1) // LSTM_CHUNK
            for k in range(nchunk):
                c0 = k * LSTM_CHUNK
                c1 = min(npc, c0 + LSTM_CHUNK)
                w = c1 - c0
                gate = {}
                for g in "igo":
                    ps = lps.tile([P, LSTM_CHUNK], dt.float32, tag="ps")
                    nc.tensor.matmul(
                        ps[:, :w], wih_t[g][:], zT_t[:, c0:c1], start=True, stop=True
                    )
                    fn = act.Tanh if g == "g" else act.Sigmoid
                    sg = ltr.tile([P, LSTM_CHUNK], dt.bfloat16, tag="sg" + g)
                    nc.scalar.activation(sg[:, :w], ps[:, :w], fn, bias=bg_t[g][:])
                    gate[g] = sg
                ct = ltr.tile([P, LSTM_CHUNK], dt.bfloat16, tag="ct")
                nc.vector.tensor_tensor(
                    ct[:, :w], gate["i"][:, :w], gate["g"][:, :w], op=alu.mult
                )
                th = ltr.tile([P, LSTM_CHUNK], dt.bfloat16, tag="th")
                nc.scalar.activation(th[:, :w], ct[:, :w], act.Tanh)
                nc.vector.tensor_tensor(
                    hT_t[:, c0:c1], gate["o"][:, :w], th[:, :w], op=alu.mult
                )

        # gather-piece pools (created after the LSTM scratch pools close)
        for chn in range(CH):
            stgp[chn] = ctx.enter_context(tc.tile_pool(name=f"stg{chn}", bufs=2))
            selp[chn] = ctx.enter_context(tc.tile_pool(name=f"sel{chn}", bufs=2))

        # -------- phase 2: m1 = h @ W1 per quarter -> bounce0 -> AG
        with (
            tc.tile_pool(name="mm_ps", bufs=2, space="PSUM") as mmps,
            tc.tile_pool(name="agg_ps", bufs=6, space="PSUM") as aps,
        ):
            for q in range(CH):
                for b in range(qb0[q], qb0[q + 1]):
                    nb = min(P, npc - b * P)
                    pm = mmps.tile([P, P], dt.float32, tag="pm")
                    nc.tensor.matmul(
                        pm[:nb, :],
                        hT_t[:, b * P : b * P + nb],
                        w1_t[:],
                        start=True,
                        stop=True,
                    )
                    nc.scalar.activation(
                        stage[:nb, b * P : (b + 1) * P], pm[:nb, :], act.Copy
                    )
                quarter_flush(0, q)
                issue_piece(0, q)

            # -------- phase 3+4: edge layer 1 (+ mm2 + AG1 interleaved)
            def post1(b, nb, pa):
                nc.scalar.activation(
                    xT_t[:, b * P : b * P + nb], pa[:, :nb], act.Relu, bias=b1_t[:]
                )
                pm = mmps.tile([P, P], dt.float32, tag="pm")
                nc.tensor.matmul(
                    pm[:nb, :],
                    xT_t[:, b * P : b * P + nb],
                    w2_t[:],
                    start=True,
                    stop=True,
                )
                nc.scalar.activation(
                    stage[:nb, b * P : (b + 1) * P], pm[:nb, :], act.Copy
                )
                for q in range(CH):
                    if b == qb0[q + 1] - 1:
                        quarter_flush(1, q)

            _edge_phase(nc, cfg, sched, 0, ptiles, issue_piece, aps, post1)

            # -------- phase 5+6: edge layer 2 (+ final Linear interleaved)
            with tc.tile_pool(name="ostage", bufs=2) as opool:
                ost = [None]

                def post2(b, nb, pa):
                    nc.vector.tensor_scalar(
                        xT_t[:, b * P : b * P + nb], pa[:, :nb], b2_t[:], None,
                        op0=alu.add,
                    )
                    g0 = (b // GBO) * GBO
                    if b == g0:
                        ot = opool.tile([P, GBO * P], dt.float32, tag="ost")
                        ost[0] = ot
                    ps = mmps.tile([P, P], dt.float32, tag="pm")
                    nc.tensor.matmul(
                        ps[:, :nb],
                        w3t_t[:],
                        xT_t[:, b * P : b * P + nb],
                        start=True,
                        stop=True,
                    )
                    nc.scalar.activation(
                        ost[0][:, (b - g0) * P : (b - g0) * P + nb],
                        ps[:, :nb],
                        act.Relu,
                        bias=b3_t[:],
                    )
                    if b == min(g0 + GBO, nblk) - 1:
                        c0 = g0 * P
                        c1 = min(npc, (g0 + GBO) * P)
                        nc.sync.dma_start(
                            out_d[:, c0:c1], ost[0][:, : c1 - c0]
                        )

                for q in range(CH):
                    issue_piece(1, q)
                _edge_phase(nc, cfg, sched, 1, ptiles, issue_piece, aps, post2)

    nc.compile()
    return nc


def _edge_phase(nc, cfg, sched, l, ptiles, issue_piece, aps, post):
    npc, nblk = cfg["npc"], cfg["nblk"]
    dt = mybir.dt
    T_run, base, ctb = sched["T_run"], sched["base"], sched["ctb"]
    consumed = [-1] * CH
    for b in range(nblk):
        nb = min(P, npc - b * P)
        pa = aps.tile([P, P], dt.float32, tag="pa")
        ntile_b = int(T_run[:, b].sum())
        done = 0
        for chn in range(CH):
            for t in range(int(T_run[chn][b])):
                gt = int(base[chn * nblk + b]) + t
                rel = gt - ctb[chn]
                pi, slot = divmod(rel, GT)
                if pi > consumed[chn]:
                    consumed[chn] = pi
                    issue_piece(l, chn)       # prefetch next piece
                stg, st = ptiles[l][chn][pi]
                nc.tensor.matmul(
                    pa[:],
                    stg[:, slot, :],
                    st[:, slot * P : (slot + 1) * P],
                    start=(done == 0),
                    stop=(done == ntile_b - 1),
                )
                done += 1
        post(b, nb, pa)


# ---------------------------------------------------------------- entry


def build(z, edge_index, W_ih, W_hh, b_ih, b_hh, W1, b1, W2, b2, W3, b3, cfg=None):
    """Host prep + trace + compile. Returns (nc, in_maps)."""
    if cfg is None:
        cfg = _cfg()
    ncores, npc = cfg["ncores"], cfg["npc"]
    z = np.asarray(z, dtype=np.float32)
    W_ih = np.asarray(W_ih, dtype=np.float32)
    b = np.asarray(b_ih, dtype=np.float32) + np.asarray(b_hh, dtype=np.float32)

    arrs, sched = _prep_edges(edge_index, cfg)
    nc = _build_nc(cfg, sched)

    gi = {"i": 0, "g": 2, "o": 3}  # torch gate order i,f,g,o (f unused: c0=0)
    common = {
        "iota": np.ascontiguousarray(
            np.tile(np.arange(P, dtype=np.float32), (P, 1))
        ).astype(bf16),
        "w1": np.asarray(W1, np.float32).astype(bf16),
        "w2": np.asarray(W2, np.float32).astype(bf16),
        "w3t": np.ascontiguousarray(np.asarray(W3, np.float32).T).astype(bf16),
        "b1": np.asarray(b1, np.float32).reshape(P, 1).copy(),
        "b2": np.asarray(b2, np.float32).reshape(P, 1).copy(),
        "b3": np.asarray(b3, np.float32).reshape(P, 1).copy(),
    }
    for g, k in gi.items():
        common[f"wih_{g}"] = np.ascontiguousarray(
            W_ih[k * P : (k + 1) * P, :].T
        ).astype(bf16)
        common[f"bg_{g}"] = b[k * P : (k + 1) * P].reshape(P, 1).copy()

    in_maps = []
    for c in range(ncores):
        idx16, dstt, nrmt = arrs[c]
        m = dict(common)
        m["zT"] = np.ascontiguousarray(z[c * npc : (c + 1) * npc].T).astype(bf16)
        m["idx16"] = idx16
        m["dstv"] = dstt
        m["nrmv"] = nrmt
        in_maps.append(m)
    return nc, in_maps


def assemble(results, cfg=None):
    if cfg is None:
        cfg = _cfg()
    ncores, npc = cfg["ncores"], cfg["npc"]
    out = np.empty((ncores * npc, P), np.float32)
    for c in range(ncores):
        out[c * npc : (c + 1) * npc] = results[c]["outT"].T
    return out


def kernel(z, edge_index, W_ih, W_hh, b_ih, b_hh, W1, b1, W2, b2, W3, b3):
    nc, in_maps = build(z, edge_index, W_ih, W_hh, b_ih, b_hh, W1, b1, W2, b2, W3, b3)
    res = run_bass_kernel_spmd(nc, in_maps, core_ids=list(range(NCORES)))
    return assemble(res.results)


# revision 35
# speedup vs baseline: 9.2156x; 3.6174x over previous
"""Trainium2 Bass kernel for nn_Decoder (GNN message passing):
LSTM(1 step) -> GCNConv -> ReLU -> GCNConv -> Linear -> ReLU on a
100K-node / 1.6M-edge graph, SPMD across 8 NeuronCores.

Strategy (dst-node sharding, v2):
- Core c owns nodes [c*12500, (c+1)*12500) and all edges into them.
- Per-node compute (LSTM, x@W transforms) runs feature-major [128, nodes].
- The transformed node table is replicated across cores via per-quarter
  AllGathers into 4 DRAM chunk tables (<=32768 rows each: int16 gather
  index limit); quarters are 128-block-aligned so the AG input is a
  contiguous slice of the node-major bounce buffer.
- GCN propagate: per dst 128-block, gather source rows (bf16) from the
  chunk tables with gpsimd.dma_gather (in-order per-chunk piece streams,
  prefetched), scatter-add via PE matmul against 128x128 selection
  matrices, accumulated in PSUM.
- Selection matrices are built in bulk, one gather piece at a time, with
  two broadcast tensor_tensor ops: st = (iota == dstv) * nrm.
- Per-block interleaving: x@W2 runs inside edge layer 1 (quarters AG as
  soon as ready); the final Linear runs inside edge layer 2 with grouped
  output DMAs.
"""

import os
from contextlib import ExitStack

import numpy as np
import ml_dtypes

ABLATE = os.environ.get("KERNEL_ABLATE", "")  # "", "sel1", "sel0", "nogather"
SINGLE_PACKET = os.environ.get("KERNEL_SP", "0") == "1"
NQUEUES = int(os.environ.get("KERNEL_NQ", "4"))
ELEM2X = os.environ.get("KERNEL_E2", "0") == "1"  # 512B elem probe (wrong data)
QRR = os.environ.get("KERNEL_QRR", "1") == "1"  # round-robin queues per piece

import concourse.bacc as bacc
import concourse.mybir as mybir
import concourse.tile as tile
from concourse.bass_utils import run_bass_kernel_spmd

P = 128
N = 100000
NCORES = 8
CH = 4              # src quarters (chunk tables)
GT = 18             # tiles (of 128 edges) per gather piece
LSTM_CHUNK = 500    # nodes per LSTM column chunk
GBO = 14            # dst blocks per output DMA group

bf16 = ml_dtypes.bfloat16
f32 = np.float32


def _cfg(n=N, ncores=NCORES):
    npc = n // ncores
    nblk = (npc + P - 1) // P
    qb = (nblk + CH - 1) // CH
    qb0 = [min(nblk, q * qb) for q in range(CH + 1)]   # quarter block starts
    q0 = [min(npc, b * P) for b in qb0]                # quarter node starts
    qs = [q0[q + 1] - q0[q] for q in range(CH)]        # quarter sizes (nodes)
    assert all(0 < ncores * s <= 32768 for s in qs)
    return dict(n=n, ncores=ncores, npc=npc, nblk=nblk, qb0=qb0, q0=q0, qs=qs)


# ---------------------------------------------------------------- host prep


def _prep_edges(edge_index, cfg):
    """Sort/pad each core's incident edges into a cross-core-uniform tile
    schedule. Returns per-core device arrays + the static schedule."""
    n, ncores, npc, nblk = cfg["n"], cfg["ncores"], cfg["npc"], cfg["nblk"]
    q0, qs = cfg["q0"], cfg["qs"]

    src = np.asarray(edge_index[0], dtype=np.int64)
    dst = np.asarray(edge_index[1], dtype=np.int64)

    # symmetric norm with self-loops in the degree; self-loop edges are NOT
    # gathered — their diagonal contribution dinv^2*(xW) is added via a
    # per-block diag matmul on the node-major mm stage
    deg = (np.bincount(dst, minlength=n) + 1).astype(np.float64)
    dinv = 1.0 / np.sqrt(deg)
    norm = (dinv[src] * dinv[dst]).astype(np.float32)

    dinv2 = np.zeros((ncores, P, nblk), np.float32)
    d2 = (dinv * dinv).astype(np.float32)
    for c in range(ncores):
        loc = d2[c * npc : (c + 1) * npc]
        pad = np.zeros(nblk * P, np.float32)
        pad[: len(loc)] = loc
        dinv2[c] = pad.reshape(nblk, P).T

    # chunk (src quarter) + row in that chunk's table
    s_rank = src // npc
    s_local = src % npc
    s_ch = np.searchsorted(np.asarray(q0[1 : CH + 1]), s_local, side="right")
    qs_a = np.asarray(qs)
    q0_a = np.asarray(q0[:CH])
    s_row = s_rank * qs_a[s_ch] + (s_local - q0_a[s_ch])

    core_of = dst // npc
    per_core = []
    counts = np.zeros((ncores, CH, nblk), np.int64)
    for c in range(ncores):
        m = core_of == c
        row, d, w, ch = s_row[m], dst[m] - c * npc, norm[m], s_ch[m]
        o = np.lexsort((d, ch))
        row, d, w, ch = row[o], d[o], w[o], ch[o]
        b = d // P
        counts[c] = np.bincount(ch * nblk + b, minlength=CH * nblk).reshape(
            CH, nblk
        )
        per_core.append((row, d, w, ch, b))

    assert counts.sum(axis=(0, 1)).min() > 0, "empty dst block"

    T_run = (counts.max(axis=0) + P - 1) // P          # [CH, nblk]
    flat = T_run.reshape(-1)
    base = np.zeros(CH * nblk + 1, np.int64)
    np.cumsum(flat, out=base[1:])                      # tile offset per run
    TT = int(base[-1])
    NIDX = TT * P
    ctb = [int(base[ch * nblk]) for ch in range(CH)] + [TT]  # chunk tile base

    arrs = []
    for c in range(ncores):
        row, d, w, ch, b = per_core[c]
        gid = ch * nblk + b
        cnt = counts[c].reshape(-1)
        gstart = np.concatenate([[0], np.cumsum(cnt)[:-1]])
        within = np.arange(len(row)) - gstart[gid]
        pos = base[gid] * P + within

        idxs = np.zeros(NIDX, np.int16)                 # pad -> row 0 (valid)
        idxs[pos] = row.astype(np.int16)
        dstv = np.full(NIDX, -1.0, np.float32)          # pad -> no dst match
        dstv[pos] = (d - b * P).astype(np.float32)
        nrmv = np.zeros(NIDX, np.float32)
        nrmv[pos] = w

        if ELEM2X:
            idxs = idxs // 2  # keep 512B-row probe reads in bounds
        idx16 = np.tile(np.ascontiguousarray(idxs.reshape(-1, 16).T), (8, 1))
        dstt = np.ascontiguousarray(dstv.reshape(TT, P).T).astype(bf16)
        nrmt = np.ascontiguousarray(nrmv.reshape(TT, P).T).astype(bf16)
        arrs.append((idx16, dstt, nrmt, dinv2[c]))

    # gather pieces: per chunk, consecutive groups of <= GT tiles
    pieces = []
    for chn in range(CH):
        t0, t1 = ctb[chn], ctb[chn + 1]
        pieces.append([(t, min(GT, t1 - t)) for t in range(t0, t1, GT)])

    sched = dict(T_run=T_run, base=base, TT=TT, NIDX=NIDX, ctb=ctb, pieces=pieces)
    return arrs, sched


# ---------------------------------------------------------------- device


def _build_nc(cfg, sched):
    ncores, npc, nblk = cfg["ncores"], cfg["npc"], cfg["nblk"]
    qb0, q0, qs = cfg["qb0"], cfg["q0"], cfg["qs"]
    T_run, base, TT, NIDX, ctb, pieces = (
        sched["T_run"],
        sched["base"],
        sched["TT"],
        sched["NIDX"],
        sched["ctb"],
        sched["pieces"],
    )
    dt = mybir.dt
    alu = mybir.AluOpType
    act = mybir.ActivationFunctionType

    nc = bacc.Bacc(
        "TRN2",
        target_bir_lowering=False,
        debug=False,
        num_devices=ncores,
        num_swdge_queues=NQUEUES,
    )

    # ---- I/O
    zT_d = nc.dram_tensor("zT", [P, npc], dt.bfloat16, kind="ExternalInput")
    idx_d = nc.dram_tensor("idx16", [P, NIDX // 16], dt.int16, kind="ExternalInput")
    dst_d = nc.dram_tensor("dstv", [P, TT], dt.bfloat16, kind="ExternalInput")
    nrm_d = nc.dram_tensor("nrmv", [P, TT], dt.bfloat16, kind="ExternalInput")
    iota_d = nc.dram_tensor("iota", [P, P], dt.bfloat16, kind="ExternalInput")
    wih_d = {
        g: nc.dram_tensor(f"wih_{g}", [P, P], dt.bfloat16, kind="ExternalInput")
        for g in "igo"
    }
    bg_d = {
        g: nc.dram_tensor(f"bg_{g}", [P, 1], dt.float32, kind="ExternalInput")
        for g in "igo"
    }
    w1_d = nc.dram_tensor("w1", [P, P], dt.bfloat16, kind="ExternalInput")
    w2_d = nc.dram_tensor("w2", [P, P], dt.bfloat16, kind="ExternalInput")
    w3t_d = nc.dram_tensor("w3t", [P, P], dt.bfloat16, kind="ExternalInput")
    b1_d = nc.dram_tensor("b1", [P, 1], dt.float32, kind="ExternalInput")
    b2_d = nc.dram_tensor("b2", [P, 1], dt.float32, kind="ExternalInput")
    b3_d = nc.dram_tensor("b3", [P, 1], dt.float32, kind="ExternalInput")
    dinv2_d = nc.dram_tensor("dinv2", [P, nblk], dt.float32, kind="ExternalInput")
    pidx_d = nc.dram_tensor("pidx", [P, 1], dt.float32, kind="ExternalInput")
    out_d = nc.dram_tensor("outT", [P, npc], dt.float32, kind="ExternalOutput")

    bounce = [nc.dram_tensor(f"bounce{l}", [npc, P], dt.bfloat16) for l in range(2)]
    table = [
        [
            nc.dram_tensor(f"table{l}_{q}", [ncores * qs[q], P], dt.bfloat16)
            for q in range(CH)
        ]
        for l in range(2)
    ]

    with tile.TileContext(nc) as tc, ExitStack() as ctx:
        konst = ctx.enter_context(tc.tile_pool(name="konst", bufs=1))
        xpool = ctx.enter_context(tc.tile_pool(name="xpool", bufs=1))
        spool = ctx.enter_context(tc.tile_pool(name="spool", bufs=1))

        def load_const(handle, shape, dtype):
            t = konst.tile(shape, dtype, tag=handle.name)
            nc.sync.dma_start(t[:], handle[:])
            return t

        iota_t = load_const(iota_d, [P, P], dt.bfloat16)
        wih_t = {g: load_const(wih_d[g], [P, P], dt.bfloat16) for g in "igo"}
        bg_t = {g: load_const(bg_d[g], [P, 1], dt.float32) for g in "igo"}
        w1_t = load_const(w1_d, [P, P], dt.bfloat16)
        w2_t = load_const(w2_d, [P, P], dt.bfloat16)
        w3t_t = load_const(w3t_d, [P, P], dt.bfloat16)
        b1_t = load_const(b1_d, [P, 1], dt.float32)
        b2_t = load_const(b2_d, [P, 1], dt.float32)
        b3_t = load_const(b3_d, [P, 1], dt.float32)
        dinv2_t = load_const(dinv2_d, [P, nblk], dt.float32)
        pidx_t = load_const(pidx_d, [P, 1], dt.float32)
        idx_t = load_const(idx_d, [P, NIDX // 16], dt.int16)
        dst_t = load_const(dst_d, [P, TT], dt.bfloat16)
        nrm_t = load_const(nrm_d, [P, TT], dt.bfloat16)

        xT_t = xpool.tile([P, npc], dt.bfloat16, tag="xT")   # x1T then x2T
        stage = spool.tile([P, nblk * P], dt.bfloat16, tag="stage")

        # per-chunk gather-piece pools (double-buffered, in-order streams);
        # created after the LSTM scratch pools close, to fit SBUF
        stgp = [None] * CH
        selp = [None] * CH
        issued = [[-1] * CH for _ in range(2)]      # last piece issued, per layer
        ptiles = [[{} for _ in range(CH)] for _ in range(2)]
        qctr = [0]

        def issue_piece(l, chn):
            k = issued[l][chn] + 1
            if k >= len(pieces[chn]):
                return
            pt0, pnt = pieces[chn][k]
            qctr[0] += 1
            stg = stgp[chn].tile([P, GT, P], dt.bfloat16, tag=f"stg{chn}")
            gnt = 1 if ABLATE == "nogather" else pnt
            if ELEM2X:
                # timing probe: 512B rows (2 table rows per index; garbage data)
                nc.gpsimd.dma_gather(
                    stg[:, : 2 * (gnt // 2), :].rearrange(
                        "p (t u) c -> p t (u c)", u=2
                    ),
                    table[l][chn][:].rearrange("(r u) c -> r (u c)", u=2),
                    idx_t[:, pt0 * 8 : pt0 * 8 + (gnt // 2) * 8],
                    (gnt // 2) * P,
                    (gnt // 2) * P,
                    2 * P,
                    single_packet=SINGLE_PACKET,
                    queue_num=(qctr[0] % NQUEUES) if QRR else (chn % NQUEUES),
                )
            else:
                nc.gpsimd.dma_gather(
                    stg[:, :gnt, :],
                    table[l][chn][:],
                    idx_t[:, pt0 * 8 : (pt0 + gnt) * 8],
                    gnt * P,
                    gnt * P,
                    P,
                    single_packet=SINGLE_PACKET,
                    queue_num=(qctr[0] % NQUEUES) if QRR else (chn % NQUEUES),
                )
            st = selp[chn].tile([P, GT * P], dt.bfloat16, tag=f"sel{chn}")
            stv = st[:, : pnt * P].rearrange("p (t c) -> p t c", c=P)
            iotaB = iota_t[:].unsqueeze(1).broadcast_to([P, pnt, P])
            dstvB = (
                dst_t[:, pt0 : pt0 + pnt].unsqueeze(2).broadcast_to([P, pnt, P])
            )
            nrmB = (
                nrm_t[:, pt0 : pt0 + pnt].unsqueeze(2).broadcast_to([P, pnt, P])
            )
            if ABLATE in ("sel0", "nogather"):
                snt = 1
                stv1 = st[:, : snt * P].rearrange("p (t c) -> p t c", c=P)
                nc.vector.tensor_tensor(
                    stv1,
                    iota_t[:].unsqueeze(1).broadcast_to([P, snt, P]),
                    dst_t[:, pt0 : pt0 + snt].unsqueeze(2).broadcast_to([P, snt, P]),
                    op=alu.is_equal,
                )
            else:
                nc.vector.tensor_tensor(stv, iotaB, dstvB, op=alu.is_equal)
                if ABLATE != "sel1":
                    nc.vector.tensor_tensor(stv, stv, nrmB, op=alu.mult)
            ptiles[l][chn][k] = (stg, st)
            issued[l][chn] = k

        def quarter_flush(l, q):
            """DMA stage quarter q -> bounce[l], then AllGather the quarter."""
            r0, r1 = q0[q], q0[q + 1]
            full = ((r1 - r0) // P) * P
            if full:
                nc.sync.dma_start(
                    bounce[l][r0 : r0 + full, :].rearrange("(b p) f -> p b f", p=P),
                    stage[:, r0 : r0 + full].rearrange("p (b f) -> p b f", f=P),
                )
            rem = (r1 - r0) - full
            if rem:
                nc.sync.dma_start(
                    bounce[l][r0 + full : r1, :],
                    stage[:rem, r0 + full : r0 + full + P],
                )
            nc.gpsimd.collective_compute(
                "AllGather",
                mybir.AluOpType.bypass,
                replica_groups=[list(range(ncores))],
                ins=[bounce[l][r0:r1, :]],
                outs=[table[l][q][:]],
            )

        # ---------------- phase 1: LSTM -> hT (feature-major, bf16)
        hpool = ctx.enter_context(tc.tile_pool(name="h_pool", bufs=1))
        hT_t = hpool.tile([P, npc], dt.bfloat16, tag="hT")
        with (
            tc.tile_pool(name="lstm_sb", bufs=1) as lsb,
            tc.tile_pool(name="lstm_ps", bufs=6, space="PSUM") as lps,
            tc.tile_pool(name="lstm_tr", bufs=8) as ltr,
        ):
            zT_t = lsb.tile([P, npc], dt.bfloat16, tag="zT")
            nc.sync.dma_start(zT_t[:], zT_d[:])

            nchunk = (npc + LSTM_CHUNK - 1) // LSTM_CHUNK
            for k in range(nchunk):
                c0 = k * LSTM_CHUNK
                c1 = min(npc, c0 + LSTM_CHUNK)
                w = c1 - c0
                gate = {}
                for g in "igo":
                    ps = lps.tile([P, LSTM_CHUNK], dt.float32, tag="ps")
                    nc.tensor.matmul(
                        ps[:, :w], wih_t[g][:], zT_t[:, c0:c1], start=True, stop=True
                    )
                    fn = act.Tanh if g == "g" else act.Sigmoid
                    sg = ltr.tile([P, LSTM_CHUNK], dt.bfloat16, tag="sg" + g)
                    nc.scalar.activation(sg[:, :w], ps[:, :w], fn, bias=bg_t[g][:])
                    gate[g] = sg
                ct = ltr.tile([P, LSTM_CHUNK], dt.bfloat16, tag="ct")
                nc.vector.tensor_tensor(
                    ct[:, :w], gate["i"][:, :w], gate["g"][:, :w], op=alu.mult
                )
                th = ltr.tile([P, LSTM_CHUNK], dt.bfloat16, tag="th")
                nc.scalar.activation(th[:, :w], ct[:, :w], act.Tanh)
                nc.vector.tensor_tensor(
                    hT_t[:, c0:c1], gate["o"][:, :w], th[:, :w], op=alu.mult
                )

        # gather-piece pools (created after the LSTM scratch pools close)
        for chn in range(CH):
            stgp[chn] = ctx.enter_context(tc.tile_pool(name=f"stg{chn}", bufs=2))
            selp[chn] = ctx.enter_context(tc.tile_pool(name=f"sel{chn}", bufs=2))

        # -------- phase 2: m1 = h @ W1 per quarter -> bounce0 -> AG
        with (
            tc.tile_pool(name="mm_ps", bufs=2, space="PSUM") as mmps,
            tc.tile_pool(name="agg_ps", bufs=6, space="PSUM") as aps,
            tc.tile_pool(name="dg_pool", bufs=2) as dgp,
        ):

            def self_term(b, nb, pa, stop0):
                # self-loop diagonal: pa += (stage_blk).T-style matmul with
                # diag(dinv^2) built from iota==partition-index
                dg = dgp.tile([P, P], dt.bfloat16, tag="dg")
                nc.vector.tensor_scalar(
                    dg[:nb, :],
                    iota_t[:nb, :],
                    pidx_t[:nb, :],
                    dinv2_t[:nb, b : b + 1],
                    op0=alu.is_equal,
                    op1=alu.mult,
                )
                nc.tensor.matmul(
                    pa[:],
                    stage[:nb, b * P : (b + 1) * P],
                    dg[:nb, :],
                    start=True,
                    stop=stop0,
                )
            for q in range(CH):
                for b in range(qb0[q], qb0[q + 1]):
                    nb = min(P, npc - b * P)
                    pm = mmps.tile([P, P], dt.float32, tag="pm")
                    nc.tensor.matmul(
                        pm[:nb, :],
                        hT_t[:, b * P : b * P + nb],
                        w1_t[:],
                        start=True,
                        stop=True,
                    )
                    nc.scalar.activation(
                        stage[:nb, b * P : (b + 1) * P], pm[:nb, :], act.Copy
                    )
                quarter_flush(0, q)
                issue_piece(0, q)

            # -------- phase 3+4: edge layer 1 (+ mm2 + AG1 interleaved)
            def post1(b, nb, pa):
                nc.scalar.activation(
                    xT_t[:, b * P : b * P + nb], pa[:, :nb], act.Relu, bias=b1_t[:]
                )
                pm = mmps.tile([P, P], dt.float32, tag="pm")
                nc.tensor.matmul(
                    pm[:nb, :],
                    xT_t[:, b * P : b * P + nb],
                    w2_t[:],
                    start=True,
                    stop=True,
                )
                nc.scalar.activation(
                    stage[:nb, b * P : (b + 1) * P], pm[:nb, :], act.Copy
                )
                for q in range(CH):
                    if b == qb0[q + 1] - 1:
                        quarter_flush(1, q)

            _edge_phase(nc, cfg, sched, 0, ptiles, issue_piece, aps, self_term, post1)

            # -------- phase 5+6: edge layer 2 (+ final Linear interleaved)
            with tc.tile_pool(name="ostage", bufs=2) as opool:
                ost = [None]

                def post2(b, nb, pa):
                    nc.vector.tensor_scalar(
                        xT_t[:, b * P : b * P + nb], pa[:, :nb], b2_t[:], None,
                        op0=alu.add,
                    )
                    g0 = (b // GBO) * GBO
                    if b == g0:
                        ot = opool.tile([P, GBO * P], dt.float32, tag="ost")
                        ost[0] = ot
                    ps = mmps.tile([P, P], dt.float32, tag="pm")
                    nc.tensor.matmul(
                        ps[:, :nb],
                        w3t_t[:],
                        xT_t[:, b * P : b * P + nb],
                        start=True,
                        stop=True,
                    )
                    nc.scalar.activation(
                        ost[0][:, (b - g0) * P : (b - g0) * P + nb],
                        ps[:, :nb],
                        act.Relu,
                        bias=b3_t[:],
                    )
                    if b == min(g0 + GBO, nblk) - 1:
                        c0 = g0 * P
                        c1 = min(npc, (g0 + GBO) * P)
                        nc.sync.dma_start(
                            out_d[:, c0:c1], ost[0][:, : c1 - c0]
                        )

                for q in range(CH):
                    issue_piece(1, q)
                _edge_phase(
                    nc, cfg, sched, 1, ptiles, issue_piece, aps, self_term, post2
                )

    nc.compile()
    return nc


def _edge_phase(nc, cfg, sched, l, ptiles, issue_piece, aps, pre, post):
    npc, nblk = cfg["npc"], cfg["nblk"]
    dt = mybir.dt
    T_run, base, ctb = sched["T_run"], sched["base"], sched["ctb"]
    consumed = [-1] * CH
    for b in range(nblk):
        nb = min(P, npc - b * P)
        pa = aps.tile([P, P], dt.float32, tag="pa")
        ntile_b = int(T_run[:, b].sum())
        pre(b, nb, pa, ntile_b == 0)  # self-loop diag matmul (start=True)
        done = 0
        for chn in range(CH):
            for t in range(int(T_run[chn][b])):
                gt = int(base[chn * nblk + b]) + t
                rel = gt - ctb[chn]
                pi, slot = divmod(rel, GT)
                if pi > consumed[chn]:
                    consumed[chn] = pi
                    issue_piece(l, chn)       # prefetch next piece
                stg, st = ptiles[l][chn][pi]
                nc.tensor.matmul(
                    pa[:],
                    stg[:, slot, :],
                    st[:, slot * P : (slot + 1) * P],
                    start=False,
                    stop=(done == ntile_b - 1),
                )
                done += 1
        post(b, nb, pa)


# ---------------------------------------------------------------- entry


def build(z, edge_index, W_ih, W_hh, b_ih, b_hh, W1, b1, W2, b2, W3, b3, cfg=None):
    """Host prep + trace + compile. Returns (nc, in_maps)."""
    if cfg is None:
        cfg = _cfg()
    ncores, npc = cfg["ncores"], cfg["npc"]
    z = np.asarray(z, dtype=np.float32)
    W_ih = np.asarray(W_ih, dtype=np.float32)
    b = np.asarray(b_ih, dtype=np.float32) + np.asarray(b_hh, dtype=np.float32)

    arrs, sched = _prep_edges(edge_index, cfg)
    nc = _build_nc(cfg, sched)

    gi = {"i": 0, "g": 2, "o": 3}  # torch gate order i,f,g,o (f unused: c0=0)
    common = {
        "iota": np.ascontiguousarray(
            np.tile(np.arange(P, dtype=np.float32), (P, 1))
        ).astype(bf16),
        "w1": np.asarray(W1, np.float32).astype(bf16),
        "w2": np.asarray(W2, np.float32).astype(bf16),
        "w3t": np.ascontiguousarray(np.asarray(W3, np.float32).T).astype(bf16),
        "b1": np.asarray(b1, np.float32).reshape(P, 1).copy(),
        "b2": np.asarray(b2, np.float32).reshape(P, 1).copy(),
        "b3": np.asarray(b3, np.float32).reshape(P, 1).copy(),
        "pidx": np.arange(P, dtype=np.float32).reshape(P, 1).copy(),
    }
    for g, k in gi.items():
        common[f"wih_{g}"] = np.ascontiguousarray(
            W_ih[k * P : (k + 1) * P, :].T
        ).astype(bf16)
        common[f"bg_{g}"] = b[k * P : (k + 1) * P].reshape(P, 1).copy()

    in_maps = []
    for c in range(ncores):
        idx16, dstt, nrmt, d2c = arrs[c]
        m = dict(common)
        m["zT"] = np.ascontiguousarray(z[c * npc : (c + 1) * npc].T).astype(bf16)
        m["idx16"] = idx16
        m["dstv"] = dstt
        m["nrmv"] = nrmt
        m["dinv2"] = np.ascontiguousarray(d2c)
        in_maps.append(m)
    return nc, in_maps


def assemble(results, cfg=None):
    if cfg is None:
        cfg = _cfg()
    ncores, npc = cfg["ncores"], cfg["npc"]
    out = np.empty((ncores * npc, P), np.float32)
    for c in range(ncores):
        out[c * npc : (c + 1) * npc] = results[c]["outT"].T
    return out


def kernel(z, edge_index, W_ih, W_hh, b_ih, b_hh, W1, b1, W2, b2, W3, b3):
    nc, in_maps = build(z, edge_index, W_ih, W_hh, b_ih, b_hh, W1, b1, W2, b2, W3, b3)
    res = run_bass_kernel_spmd(nc, in_maps, core_ids=list(range(NCORES)))
    return assemble(res.results)
